# revision 5
# baseline (speedup 1.0000x reference)
"""Trainium2 Bass kernel for nn_Cell2Tissue (scatter_memory) — v4.

vs v3 (148 us): the conv re-shards 8-way over full channels and the tail
collapses into an accumulate-DMA.
- each core computes ALL 128 output channels for avg columns [8c, 8c+8)
  (vs 64 channels x 16 columns): the PE runs the full 128-wide array
  (matmul time halves to ~13 us busy, 36 matmuls into one PSUM bank),
  the cell slice halves to 2.4 MB (9 plane cols vs 17), and one 8-way
  AllGather over all cores shares the full-channel blocks. The core's
  channel half is selected at readback with a dynamic offset input.
- cell tiles load on the two HWDGE queues ahead of the copy (v3 put them
  on gpsimd, whose 2 KB descriptors got ~6% of the SDMA round-robin and
  starved the conv until 66 us).
- the ROI update is a SWDGE accumulate-DMA (out_roi += avgT, CCE add):
  no early ROI read, no DVE add, tail = barrier + 2 accum DMAs.
- bulk copy unchanged from v3: bf16 through SBUF, 4x [128, 8192] chunks,
  loads on scalar / stores on sync (16 KB per-partition descriptors).
"""

import os
import numpy as np

B, C, H, W = 4, 128, 256, 256
CH = C // 2          # channels per output-shard half
L = 32               # half ROI width
ROI = 2 * L          # 64
NCORES = 8
PRR = 65             # polyphase plane rows (max y+pb = 64)
PRC = 66             # polyphase plane cols
PHASES = 16
BCOLS = 8            # avg columns computed per core
KPR = BCOLS + 1      # plane cols needed per core

_CACHE = {}


def _get_modules():
    if "mods" in _CACHE:
        return _CACHE["mods"]
    if os.environ.get("JAX_PLATFORMS") in ("cpu",):
        del os.environ["JAX_PLATFORMS"]
    import concourse.bass as bass
    import concourse.mybir as mybir
    import concourse.tile as tile
    from concourse.bass_utils import run_bass_kernel_spmd

    _CACHE["mods"] = (bass, mybir, tile, run_bass_kernel_spmd)
    return _CACHE["mods"]


def _split_multiwaits(nc, mybir, max_waits=1):
    """The walrus build here rejects >1 sem-wait on some instructions (the
    Tile tail InstDrain, DMA_DIRECT2D). Hoist extra waits onto single-wait
    nops placed immediately before, on the same engine (same-engine program
    order preserves semantics)."""
    for fn in nc.m.functions:
        for bb in fn.blocks:
            insts = bb.instructions
            i = 0
            while i < len(insts):
                inst = insts[i]
                si = inst.sync_info
                if si is not None and si.on_wait and len(si.on_wait) > max_waits:
                    waits = list(si.on_wait)
                    keep = waits[-max_waits:]
                    for k, w in enumerate(waits[:-max_waits]):
                        nop = mybir.InstNoOp(
                            name=f"{inst.name}_hoistwait_{k}",
                            sync_info=mybir.SyncInfo(on_wait=[w], on_update=[]),
                            bass_nofuse=True,
                            engine=inst.engine,
                        )
                        insts.insert(i, nop)
                        i += 1
                    si.on_wait = keep
                i += 1


def _build_program():
    """Per-core inputs
      tissue (256, 256*64) bf16 channel-last (sample j, channel half h),
      cellrows (128,16,65,9) bf16 (plane-col slice for avg cols [8c,8c+8)),
      w6t (128,36,128) bf16 (full), bias (128,1) f32 (full),
      roff (1,3) i32 = [row0, col0, 512*h]
    output: out (256, 256*64) bf16 = tissue with avg added in the ROI."""
    if "nc" in _CACHE:
        return _CACHE["nc"]
    bass, mybir, tile, _ = _get_modules()
    f32, bf16, i32 = mybir.dt.float32, mybir.dt.bfloat16, mybir.dt.int32

    nc = bass.Bass("TRN2", target_bir_lowering=False, debug=False,
                   num_devices=NCORES)
    tissue_d = nc.dram_tensor("tissue", (H, W * CH), bf16, kind="ExternalInput").ap()
    cellrows_d = nc.dram_tensor("cellrows", (C, PHASES, PRR, KPR), bf16,
                                kind="ExternalInput").ap()
    w6t_d = nc.dram_tensor("w6t", (C, 36, C), bf16, kind="ExternalInput").ap()
    bias_d = nc.dram_tensor("bias", (C, 1), f32, kind="ExternalInput").ap()
    roff_d = nc.dram_tensor("roff", (1, 3), i32, kind="ExternalInput").ap()
    out_d = nc.dram_tensor("out", (H, W * CH), bf16, kind="ExternalOutput").ap()

    # taps grouped by polyphase plane group so matmuls chase the tile DMAs
    tap_order = []       # (tap_idx, plane, row_shift, col_shift)
    for pp in range(4):
        for qq in range(4):
            for p in range(pp, 6, 4):
                for q in range(qq, 6, 4):
                    tap_order.append((p * 6 + q, pp * 4 + qq, p // 4, q // 4))
    assert len(tap_order) == 36

    with tile.TileContext(nc) as tc:
        with (
            tc.tile_pool(name="const", bufs=1) as constp,
            tc.tile_pool(name="cellp", bufs=1) as cellp,
            tc.tile_pool(name="roip", bufs=1) as roip,
            tc.tile_pool(name="copyp", bufs=3) as copyp,
            tc.tile_pool(name="avgp", bufs=1) as avgp,
            tc.tile_pool(name="dram", bufs=1, space="DRAM") as dramp,
            tc.tile_pool(name="psum", bufs=1, space="PSUM") as psump,
        ):
            # --- tiny constants on gpsimd; weights first on scalar ---
            roff_sb = constp.tile([1, 3], i32)
            nc.gpsimd.dma_start(roff_sb[:], roff_d[:])
            bias_sb = constp.tile([C, 1], f32)
            nc.gpsimd.dma_start(bias_sb[:], bias_d[:])
            w_sb = constp.tile([C, 36 * C], bf16)
            nc.scalar.dma_start(w_sb[:], w6t_d[:])

            # --- cell planes: 4 plane-group tiles, 2 per HWDGE queue,
            # ahead of the bulk copy ---
            cr_ts = []
            cd4 = cellrows_d.rearrange("c (g p) r w -> g c (p r w)", g=4)
            for g in range(4):
                crt = cellp.tile([C, (PHASES // 4) * PRR * KPR], bf16,
                                 name=f"cr{g}")
                (nc.scalar if g < 2 else nc.sync).dma_start(crt[:], cd4[g])
                cr_ts.append(crt.rearrange("c (ph r w) -> c ph r w",
                                           r=PRR, w=KPR))
            zero_sb = constp.tile([C, 512], f32)
            nc.gpsimd.memset(zero_sb[:], 0.0)

            dyn_engines = (mybir.EngineType.SP, mybir.EngineType.Activation,
                           mybir.EngineType.Pool)
            r_v = nc.values_load(roff_sb[0:1, 0:1], engines=dyn_engines,
                                 min_val=0, max_val=H - ROI,
                                 skip_runtime_bounds_check=True)
            c_v = nc.values_load(roff_sb[0:1, 1:2], engines=dyn_engines,
                                 min_val=0, max_val=W - ROI,
                                 skip_runtime_bounds_check=True)
            h_v = nc.values_load(roff_sb[0:1, 2:3], engines=dyn_engines,
                                 min_val=0, max_val=CH * BCOLS,
                                 skip_runtime_bounds_check=True)

            tissue3 = tissue_d.rearrange("h (w c) -> h w c", c=CH)
            out3 = out_d.rearrange("h (w c) -> h w c", c=CH)

            # ROI source pixels, [128, 2048] layout (partition = r + 64*b,
            # b = column half): read early on scalar's ring
            roi_src = tissue3[bass.ds(r_v, ROI), bass.ds(c_v, ROI), :]
            roiT_sb = roip.tile([2 * ROI, (ROI // 2) * CH], bf16)
            for b in range(2):
                nc.scalar.dma_start(
                    roiT_sb[64 * b:64 * b + 64].rearrange(
                        "r (w c) -> r w c", c=CH),
                    roi_src[:, 32 * b:32 * b + 32, :],
                )

            # --- bulk copy tissue -> out through SBUF, bf16 ---
            CK = (W * CH) // 2
            for s in range(2):
                for k in range(2):
                    ct = copyp.tile([128, CK], bf16, tag="cp")
                    nc.scalar.dma_start(
                        ct[:], tissue_d[s * 128:(s + 1) * 128,
                                        k * CK:(k + 1) * CK])
                    nc.sync.dma_start(
                        out_d[s * 128:(s + 1) * 128, k * CK:(k + 1) * CK],
                        ct[:])

            # --- conv: full-channel avg cols [8c, 8c+8), 36 matmuls into
            # one PSUM bank, full 128-wide array ---
            ps = psump.tile([C, 64 * BCOLS], f32, name="bank0")
            for i, (t, ph, pb, qb) in enumerate(tap_order):
                nc.tensor.matmul(
                    ps[:],
                    w_sb[:, t * C:(t + 1) * C],
                    cr_ts[ph // 4][:, ph % 4, pb:pb + 64, qb:qb + BCOLS],
                    start=(i == 0),
                    stop=(i == 35),
                )

            # psum -> SBUF -> bf16; bias folded in (DVE)
            contrib_sb = avgp.tile([C, 64 * BCOLS], bf16)
            nc.vector.scalar_tensor_tensor(
                contrib_sb[:], ps[:], bias_sb[:], zero_sb[:],
                mybir.AluOpType.add, mybir.AluOpType.add,
            )

            # transpose [ch, (r64, c8)] -> contribT [r, (h2, c8, chh64)]
            # (bounce layout groups each channel half contiguously so the
            # dynamic-half readback is one 512-elem slice per block)
            contribT_sb = avgp.tile([ROI, 2 * BCOLS * CH], bf16)
            cv = contrib_sb.rearrange("p (r c) -> p c r", c=BCOLS)
            atv = contribT_sb.rearrange("p (h c q) -> p h c q", h=2, c=BCOLS)
            for bi in range(4):      # source ch 32-block
                for bj in range(2):  # source row 32-block
                    nc.vector.transpose(
                        atv[32 * bj:32 * bj + 32, bi // 2, :,
                            32 * (bi % 2):32 * (bi % 2) + 32],
                        cv[32 * bi:32 * bi + 32, :, 32 * bj:32 * bj + 32],
                    )
            bounce_in = dramp.tile([ROI, 2 * BCOLS * CH], bf16)
            nc.gpsimd.dma_start(bounce_in[:], contribT_sb[:])

            # --- AllGather the 8 column blocks across all cores (bf16) ---
            bounce_out = dramp.tile([8 * ROI, 2 * BCOLS * CH], bf16)
            nc.gpsimd.collective_compute(
                "AllGather",
                mybir.AluOpType.bypass,
                replica_groups=[[0, 1, 2, 3, 4, 5, 6, 7]],
                ins=[bounce_in[:].opt()],
                outs=[bounce_out[:].opt()],
            )
            # readback block c, this core's channel half (dynamic h_v):
            # -> avgT [128, 2048], partition = r + 64*(w>>5),
            #    free = (w&31)*64 + chh
            avgT_sb = avgp.tile([2 * ROI, (ROI // 2) * CH], bf16)
            bo3 = bounce_out.rearrange("(n p) f -> n p f", n=8)
            for c in range(8):
                eng = nc.sync if c % 2 == 0 else nc.scalar
                eng.dma_start(
                    avgT_sb[64 * (c // 4):64 * (c // 4) + 64,
                            512 * (c % 4):512 * (c % 4) + 512],
                    bo3[c][:, bass.ds(h_v, CH * BCOLS)])

            # roiT += avgT  (DVE, bf16, 128 partitions — before the barrier)
            nc.vector.scalar_tensor_tensor(
                roiT_sb[:], avgT_sb[:], 0.0, roiT_sb[:],
                mybir.AluOpType.add, mybir.AluOpType.add,
            )

            # --- ROI overwrite: both writes on sync's ring, queued
            # after its bulk stores — the FIFO ring orders stores before
            # the overwrite without an all-engine barrier (and without
            # the ~3 us second-engine skew) ---
            roi_dst = out3[bass.ds(r_v, ROI), bass.ds(c_v, ROI), :]
            for b in range(2):
                nc.sync.dma_start(
                    roi_dst[:, 32 * b:32 * b + 32, :],
                    roiT_sb[64 * b:64 * b + 64].rearrange(
                        "r (w c) -> r w c", c=CH),
                )

    _split_multiwaits(nc, mybir)
    _CACHE["nc"] = nc
    return nc


def _prep_inputs(tissue_features, cell_features, loc, conv_w, conv_b):
    import ml_dtypes

    bf16 = ml_dtypes.bfloat16
    # fold AvgPool4x4 into the conv kernel: 6x6 taps
    w6 = np.zeros((C, C, 6, 6), np.float32)
    for dr in range(4):
        for dc in range(4):
            w6[:, :, dr:dr + 3, dc:dc + 3] += conv_w
    w6 *= 1.0 / 16.0

    # polyphase split of the zero-padded cell map:
    # plane (pp,qq)[y,x] = padded[4y+pp, 4x+qq], padded = 1px zero border
    padc = np.zeros((C, 4 * PRC, 4 * PRC), np.float32)
    padc[:, 1:1 + H, 1:1 + W] = cell_features[0]
    cell_poly = np.empty((C, PHASES, PRR, PRC), np.float32)
    for pp in range(4):
        for qq in range(4):
            cell_poly[:, pp * 4 + qq] = padc[:, pp:pp + 4 * PRR:4, qq::4]
    cell_poly = cell_poly.astype(bf16)

    # (C, C, 6, 6) -> (in ch, tap, out ch), full channel set
    w6t = np.ascontiguousarray(
        w6.transpose(1, 2, 3, 0).reshape(C, 36, C)).astype(bf16)
    bias = np.ascontiguousarray(conv_b.astype(np.float32)).reshape(C, 1)

    r0 = loc[:, 1].astype(np.int64) * W // 1024 - L   # H-dim start (from loc x)
    c0 = loc[:, 0].astype(np.int64) * W // 1024 - L   # W-dim start (from loc y)

    in_maps = []
    for c in range(NCORES):
        j, h = c % B, c // B
        # channel-last [H, W*CH] bf16 layout for tissue
        thwc = np.ascontiguousarray(
            tissue_features[j, CH * h:CH * (h + 1)].transpose(1, 2, 0)
        ).astype(bf16).reshape(H, W * CH)
        in_maps.append({
            "tissue": thwc,
            "cellrows": np.ascontiguousarray(
                cell_poly[:, :, :, BCOLS * c:BCOLS * c + KPR]),
            "w6t": w6t,
            "bias": bias,
            "roff": np.array([[r0[j], c0[j], CH * BCOLS * h]], np.int32),
        })
    return in_maps


def run_device(tissue_features, cell_features, loc, conv_w, conv_b, **spmd_kwargs):
    """Build+run the SPMD kernel; returns (final (4,128,256,256), raw results)."""
    *_, run_bass_kernel_spmd = _get_modules()
    nc = _build_program()
    in_maps = _prep_inputs(tissue_features, cell_features, loc, conv_w, conv_b)
    res = run_bass_kernel_spmd(nc, in_maps, list(range(NCORES)), **spmd_kwargs)
    final = np.empty((B, C, H, W), np.float32)
    for c in range(NCORES):
        j, h = c % B, c // B
        final[j, CH * h:CH * (h + 1)] = (
            res.results[c]["out"].astype(np.float32)
            .reshape(H, W, CH).transpose(2, 0, 1))
    return final, res


def kernel(tissue_features, cell_features, loc, conv_w, conv_b):
    final, _ = run_device(tissue_features, cell_features, loc, conv_w, conv_b)
    # reference stacks B copies of the fully-mutated tissue
    return np.broadcast_to(final[None], (B, B, C, H, W))


# revision 6
# speedup vs baseline: 1.2245x; 1.2245x over previous
"""Trainium2 Bass kernel for nn_Cell2Tissue (scatter_memory).

Measured ~97-139 us HW exec (vs 152 us f32 baseline; the spread is the
runtime's pre-collective device barrier, 24-67 us of run-to-run noise
that gates the AllGather).

Sharding: core c = (sample j = c % 4, channel half h = c // 4). Design:
- bulk tissue copy in bf16 (host casts in/out; rel err ~1.7e-3 vs the
  2e-2 gate), halving the dominant HBM traffic. Through-SBUF, 4x
  [128, 8192] chunks, loads on scalar / stores on sync (16 KB
  per-partition descriptors share the SDMA round-robin fairly).
- conv shards 8-way over FULL channels: each core computes all 128
  output channels of avg cols [8c, 8c+8) (36 matmuls, one PSUM bank,
  full 128-wide PE array; 9-plane-col cell slice = 2.4 MB). Weights
  first on scalar's HWDGE queue; cell plane-group tiles split across
  both HWDGE queues ahead of the copy (small DMAs on gpsimd's SWDGE
  queue get ~6% of the round-robin and starve).
- one 8-way AllGather (DRAM bounce, bf16) shares the blocks; each core
  reads back its channel half via dynamic offset h_v with 8 parallel
  readbacks on the two HWDGE rings, then adds into the pre-read ROI
  tile on DVE ([128, 2048] layout, partition = row + 64*colhalf).
- the dynamic ROI overwrite issues BOTH writes on sync's ring, queued
  after its bulk stores: the FIFO ring orders stores-before-overwrite
  in hardware, replacing the all-engine barrier (and its ~3 us
  second-engine skew).
- dead ends (measured): DRAM->DRAM copy and big packets starve the
  collective/barrier packets; remote_dma* would remove the collective
  entirely but this walrus build cannot encode those ISA ops ("ISA
  wrong length"); Shared-space bounce buffers route ~70 us slower.
  The x4 output stack is a zero-copy host broadcast.
"""

import os
import numpy as np

B, C, H, W = 4, 128, 256, 256
CH = C // 2          # channels per output-shard half
L = 32               # half ROI width
ROI = 2 * L          # 64
NCORES = 8
PRR = 65             # polyphase plane rows (max y+pb = 64)
PRC = 66             # polyphase plane cols
PHASES = 16
BCOLS = 8            # avg columns computed per core
KPR = BCOLS + 1      # plane cols needed per core

_CACHE = {}


def _get_modules():
    if "mods" in _CACHE:
        return _CACHE["mods"]
    if os.environ.get("JAX_PLATFORMS") in ("cpu",):
        del os.environ["JAX_PLATFORMS"]
    import concourse.bass as bass
    import concourse.mybir as mybir
    import concourse.tile as tile
    from concourse.bass_utils import run_bass_kernel_spmd

    _CACHE["mods"] = (bass, mybir, tile, run_bass_kernel_spmd)
    return _CACHE["mods"]


def _split_multiwaits(nc, mybir, max_waits=1):
    """The walrus build here rejects >1 sem-wait on some instructions (the
    Tile tail InstDrain, DMA_DIRECT2D). Hoist extra waits onto single-wait
    nops placed immediately before, on the same engine (same-engine program
    order preserves semantics)."""
    for fn in nc.m.functions:
        for bb in fn.blocks:
            insts = bb.instructions
            i = 0
            while i < len(insts):
                inst = insts[i]
                si = inst.sync_info
                if si is not None and si.on_wait and len(si.on_wait) > max_waits:
                    waits = list(si.on_wait)
                    keep = waits[-max_waits:]
                    for k, w in enumerate(waits[:-max_waits]):
                        nop = mybir.InstNoOp(
                            name=f"{inst.name}_hoistwait_{k}",
                            sync_info=mybir.SyncInfo(on_wait=[w], on_update=[]),
                            bass_nofuse=True,
                            engine=inst.engine,
                        )
                        insts.insert(i, nop)
                        i += 1
                    si.on_wait = keep
                i += 1


def _build_program():
    """Per-core inputs
      tissue (256, 256*64) bf16 channel-last (sample j, channel half h),
      cellrows (128,16,65,9) bf16 (plane-col slice for avg cols [8c,8c+8)),
      w6t (128,36,128) bf16 (full), bias (128,1) f32 (full),
      roff (1,3) i32 = [row0, col0, 512*h]
    output: out (256, 256*64) bf16 = tissue with avg added in the ROI."""
    if "nc" in _CACHE:
        return _CACHE["nc"]
    bass, mybir, tile, _ = _get_modules()
    f32, bf16, i32 = mybir.dt.float32, mybir.dt.bfloat16, mybir.dt.int32

    nc = bass.Bass("TRN2", target_bir_lowering=False, debug=False,
                   num_devices=NCORES)
    tissue_d = nc.dram_tensor("tissue", (H, W * CH), bf16, kind="ExternalInput").ap()
    cellrows_d = nc.dram_tensor("cellrows", (C, PHASES, PRR, KPR), bf16,
                                kind="ExternalInput").ap()
    w6t_d = nc.dram_tensor("w6t", (C, 36, C), bf16, kind="ExternalInput").ap()
    bias_d = nc.dram_tensor("bias", (C, 1), f32, kind="ExternalInput").ap()
    roff_d = nc.dram_tensor("roff", (1, 3), i32, kind="ExternalInput").ap()
    out_d = nc.dram_tensor("out", (H, W * CH), bf16, kind="ExternalOutput").ap()

    # taps grouped by polyphase plane group so matmuls chase the tile DMAs
    tap_order = []       # (tap_idx, plane, row_shift, col_shift)
    for pp in range(4):
        for qq in range(4):
            for p in range(pp, 6, 4):
                for q in range(qq, 6, 4):
                    tap_order.append((p * 6 + q, pp * 4 + qq, p // 4, q // 4))
    assert len(tap_order) == 36

    with tile.TileContext(nc) as tc:
        with (
            tc.tile_pool(name="const", bufs=1) as constp,
            tc.tile_pool(name="cellp", bufs=1) as cellp,
            tc.tile_pool(name="roip", bufs=1) as roip,
            tc.tile_pool(name="copyp", bufs=3) as copyp,
            tc.tile_pool(name="avgp", bufs=1) as avgp,
            tc.tile_pool(name="dram", bufs=1, space="DRAM") as dramp,
            tc.tile_pool(name="psum", bufs=1, space="PSUM") as psump,
        ):
            # --- tiny constants on gpsimd; weights first on scalar ---
            roff_sb = constp.tile([1, 3], i32)
            nc.gpsimd.dma_start(roff_sb[:], roff_d[:])
            bias_sb = constp.tile([C, 1], f32)
            nc.gpsimd.dma_start(bias_sb[:], bias_d[:])
            w_sb = constp.tile([C, 36 * C], bf16)
            nc.scalar.dma_start(w_sb[:], w6t_d[:])

            # --- cell planes: 4 plane-group tiles, 2 per HWDGE queue,
            # ahead of the bulk copy ---
            cr_ts = []
            cd4 = cellrows_d.rearrange("c (g p) r w -> g c (p r w)", g=4)
            for g in range(4):
                crt = cellp.tile([C, (PHASES // 4) * PRR * KPR], bf16,
                                 name=f"cr{g}")
                (nc.scalar if g < 2 else nc.sync).dma_start(crt[:], cd4[g])
                cr_ts.append(crt.rearrange("c (ph r w) -> c ph r w",
                                           r=PRR, w=KPR))
            zero_sb = constp.tile([C, 512], f32)
            nc.gpsimd.memset(zero_sb[:], 0.0)

            dyn_engines = (mybir.EngineType.SP, mybir.EngineType.Activation,
                           mybir.EngineType.Pool)
            r_v = nc.values_load(roff_sb[0:1, 0:1], engines=dyn_engines,
                                 min_val=0, max_val=H - ROI,
                                 skip_runtime_bounds_check=True)
            c_v = nc.values_load(roff_sb[0:1, 1:2], engines=dyn_engines,
                                 min_val=0, max_val=W - ROI,
                                 skip_runtime_bounds_check=True)
            h_v = nc.values_load(roff_sb[0:1, 2:3], engines=dyn_engines,
                                 min_val=0, max_val=CH * BCOLS,
                                 skip_runtime_bounds_check=True)

            tissue3 = tissue_d.rearrange("h (w c) -> h w c", c=CH)
            out3 = out_d.rearrange("h (w c) -> h w c", c=CH)

            # ROI source pixels, [128, 2048] layout (partition = r + 64*b,
            # b = column half): read early on scalar's ring
            roi_src = tissue3[bass.ds(r_v, ROI), bass.ds(c_v, ROI), :]
            roiT_sb = roip.tile([2 * ROI, (ROI // 2) * CH], bf16)
            for b in range(2):
                nc.scalar.dma_start(
                    roiT_sb[64 * b:64 * b + 64].rearrange(
                        "r (w c) -> r w c", c=CH),
                    roi_src[:, 32 * b:32 * b + 32, :],
                )

            # --- bulk copy tissue -> out through SBUF, bf16 ---
            CK = (W * CH) // 2
            for s in range(2):
                for k in range(2):
                    ct = copyp.tile([128, CK], bf16, tag="cp")
                    nc.scalar.dma_start(
                        ct[:], tissue_d[s * 128:(s + 1) * 128,
                                        k * CK:(k + 1) * CK])
                    nc.sync.dma_start(
                        out_d[s * 128:(s + 1) * 128, k * CK:(k + 1) * CK],
                        ct[:])

            # --- conv: full-channel avg cols [8c, 8c+8), 36 matmuls into
            # one PSUM bank, full 128-wide array ---
            ps = psump.tile([C, 64 * BCOLS], f32, name="bank0")
            for i, (t, ph, pb, qb) in enumerate(tap_order):
                nc.tensor.matmul(
                    ps[:],
                    w_sb[:, t * C:(t + 1) * C],
                    cr_ts[ph // 4][:, ph % 4, pb:pb + 64, qb:qb + BCOLS],
                    start=(i == 0),
                    stop=(i == 35),
                )

            # psum -> SBUF -> bf16; bias folded in (DVE)
            contrib_sb = avgp.tile([C, 64 * BCOLS], bf16)
            nc.vector.scalar_tensor_tensor(
                contrib_sb[:], ps[:], bias_sb[:], zero_sb[:],
                mybir.AluOpType.add, mybir.AluOpType.add,
            )

            # transpose [ch, (r64, c8)] -> contribT [r, (h2, c8, chh64)]
            # (bounce layout groups each channel half contiguously so the
            # dynamic-half readback is one 512-elem slice per block)
            contribT_sb = avgp.tile([ROI, 2 * BCOLS * CH], bf16)
            cv = contrib_sb.rearrange("p (r c) -> p c r", c=BCOLS)
            atv = contribT_sb.rearrange("p (h c q) -> p h c q", h=2, c=BCOLS)
            for bi in range(4):      # source ch 32-block
                for bj in range(2):  # source row 32-block
                    nc.vector.transpose(
                        atv[32 * bj:32 * bj + 32, bi // 2, :,
                            32 * (bi % 2):32 * (bi % 2) + 32],
                        cv[32 * bi:32 * bi + 32, :, 32 * bj:32 * bj + 32],
                    )
            bounce_in = dramp.tile([ROI, 2 * BCOLS * CH], bf16)
            nc.gpsimd.dma_start(bounce_in[:], contribT_sb[:])

            # --- AllGather the 8 column blocks across all cores (bf16) ---
            bounce_out = dramp.tile([8 * ROI, 2 * BCOLS * CH], bf16)
            nc.gpsimd.collective_compute(
                "AllGather",
                mybir.AluOpType.bypass,
                replica_groups=[[0, 1, 2, 3, 4, 5, 6, 7]],
                ins=[bounce_in[:].opt()],
                outs=[bounce_out[:].opt()],
            )
            # readback block c, this core's channel half (dynamic h_v):
            # -> avgT [128, 2048], partition = r + 64*(w>>5),
            #    free = (w&31)*64 + chh
            avgT_sb = avgp.tile([2 * ROI, (ROI // 2) * CH], bf16)
            bo3 = bounce_out.rearrange("(n p) f -> n p f", n=8)
            for c in range(8):
                eng = nc.sync if c % 2 == 0 else nc.scalar
                eng.dma_start(
                    avgT_sb[64 * (c // 4):64 * (c // 4) + 64,
                            512 * (c % 4):512 * (c % 4) + 512],
                    bo3[c][:, bass.ds(h_v, CH * BCOLS)])

            # roiT += avgT  (DVE, bf16, 128 partitions — before the barrier)
            nc.vector.scalar_tensor_tensor(
                roiT_sb[:], avgT_sb[:], 0.0, roiT_sb[:],
                mybir.AluOpType.add, mybir.AluOpType.add,
            )

            # --- ROI overwrite: both writes on sync's ring, queued
            # after its bulk stores — the FIFO ring orders stores before
            # the overwrite without an all-engine barrier (and without
            # the ~3 us second-engine skew) ---
            roi_dst = out3[bass.ds(r_v, ROI), bass.ds(c_v, ROI), :]
            for b in range(2):
                nc.sync.dma_start(
                    roi_dst[:, 32 * b:32 * b + 32, :],
                    roiT_sb[64 * b:64 * b + 64].rearrange(
                        "r (w c) -> r w c", c=CH),
                )

    _split_multiwaits(nc, mybir)
    _CACHE["nc"] = nc
    return nc


def _prep_inputs(tissue_features, cell_features, loc, conv_w, conv_b):
    import ml_dtypes

    bf16 = ml_dtypes.bfloat16
    # fold AvgPool4x4 into the conv kernel: 6x6 taps
    w6 = np.zeros((C, C, 6, 6), np.float32)
    for dr in range(4):
        for dc in range(4):
            w6[:, :, dr:dr + 3, dc:dc + 3] += conv_w
    w6 *= 1.0 / 16.0

    # polyphase split of the zero-padded cell map:
    # plane (pp,qq)[y,x] = padded[4y+pp, 4x+qq], padded = 1px zero border
    padc = np.zeros((C, 4 * PRC, 4 * PRC), np.float32)
    padc[:, 1:1 + H, 1:1 + W] = cell_features[0]
    cell_poly = np.empty((C, PHASES, PRR, PRC), np.float32)
    for pp in range(4):
        for qq in range(4):
            cell_poly[:, pp * 4 + qq] = padc[:, pp:pp + 4 * PRR:4, qq::4]
    cell_poly = cell_poly.astype(bf16)

    # (C, C, 6, 6) -> (in ch, tap, out ch), full channel set
    w6t = np.ascontiguousarray(
        w6.transpose(1, 2, 3, 0).reshape(C, 36, C)).astype(bf16)
    bias = np.ascontiguousarray(conv_b.astype(np.float32)).reshape(C, 1)

    r0 = loc[:, 1].astype(np.int64) * W // 1024 - L   # H-dim start (from loc x)
    c0 = loc[:, 0].astype(np.int64) * W // 1024 - L   # W-dim start (from loc y)

    in_maps = []
    for c in range(NCORES):
        j, h = c % B, c // B
        # channel-last [H, W*CH] bf16 layout for tissue
        thwc = np.ascontiguousarray(
            tissue_features[j, CH * h:CH * (h + 1)].transpose(1, 2, 0)
        ).astype(bf16).reshape(H, W * CH)
        in_maps.append({
            "tissue": thwc,
            "cellrows": np.ascontiguousarray(
                cell_poly[:, :, :, BCOLS * c:BCOLS * c + KPR]),
            "w6t": w6t,
            "bias": bias,
            "roff": np.array([[r0[j], c0[j], CH * BCOLS * h]], np.int32),
        })
    return in_maps


def run_device(tissue_features, cell_features, loc, conv_w, conv_b, **spmd_kwargs):
    """Build+run the SPMD kernel; returns (final (4,128,256,256), raw results)."""
    *_, run_bass_kernel_spmd = _get_modules()
    nc = _build_program()
    in_maps = _prep_inputs(tissue_features, cell_features, loc, conv_w, conv_b)
    res = run_bass_kernel_spmd(nc, in_maps, list(range(NCORES)), **spmd_kwargs)
    final = np.empty((B, C, H, W), np.float32)
    for c in range(NCORES):
        j, h = c % B, c // B
        final[j, CH * h:CH * (h + 1)] = (
            res.results[c]["out"].astype(np.float32)
            .reshape(H, W, CH).transpose(2, 0, 1))
    return final, res


def kernel(tissue_features, cell_features, loc, conv_w, conv_b):
    final, _ = run_device(tissue_features, cell_features, loc, conv_w, conv_b)
    # reference stacks B copies of the fully-mutated tissue
    return np.broadcast_to(final[None], (B, B, C, H, W))


# revision 7
# speedup vs baseline: 1.2400x; 1.0126x over previous
"""Trainium2 Bass kernel for nn_Cell2Tissue (scatter_memory).

Measured ~97-139 us HW exec (vs 152 us f32 baseline; the spread is the
runtime's pre-collective device barrier, 24-67 us of run-to-run noise
that gates the AllGather).

Sharding: core c = (sample j = c % 4, channel half h = c // 4). Design:
- bulk tissue copy in bf16 (host casts in/out; rel err ~1.7e-3 vs the
  2e-2 gate), halving the dominant HBM traffic. Through-SBUF, 4x
  [128, 8192] chunks, loads on scalar / stores on sync (16 KB
  per-partition descriptors share the SDMA round-robin fairly).
- conv shards 8-way over FULL channels: each core computes all 128
  output channels of avg cols [8c, 8c+8) (36 matmuls, one PSUM bank,
  full 128-wide PE array; 9-plane-col cell slice = 2.4 MB). Weights
  first on scalar's HWDGE queue; cell plane-group tiles split across
  both HWDGE queues ahead of the copy (small DMAs on gpsimd's SWDGE
  queue get ~6% of the round-robin and starve).
- one 8-way AllGather (DRAM bounce, bf16) shares the blocks; each core
  reads back its channel half via dynamic offset h_v with ONE readback
  per HWDGE ring (split by partition half), then one full-width DVE add
  into the pre-read ROI tile ([128, 2048] layout, partition =
  row + 64*colhalf). Post-collective tail: ~12 us.
- the dynamic ROI overwrite issues BOTH writes on sync's ring, queued
  after its bulk stores: the FIFO ring orders stores-before-overwrite
  in hardware, replacing the all-engine barrier (and its ~3 us
  second-engine skew).
- dead ends (measured): DRAM->DRAM copy and big packets starve the
  collective/barrier packets; remote_dma* would remove the collective
  entirely but this walrus build cannot encode those ISA ops ("ISA
  wrong length"); Shared-space bounce buffers route ~70 us slower.
  The x4 output stack is a zero-copy host broadcast.
"""

import os
import numpy as np

B, C, H, W = 4, 128, 256, 256
CH = C // 2          # channels per output-shard half
L = 32               # half ROI width
ROI = 2 * L          # 64
NCORES = 8
PRR = 65             # polyphase plane rows (max y+pb = 64)
PRC = 66             # polyphase plane cols
PHASES = 16
BCOLS = 8            # avg columns computed per core
KPR = BCOLS + 1      # plane cols needed per core

_CACHE = {}


def _get_modules():
    if "mods" in _CACHE:
        return _CACHE["mods"]
    if os.environ.get("JAX_PLATFORMS") in ("cpu",):
        del os.environ["JAX_PLATFORMS"]
    import concourse.bass as bass
    import concourse.mybir as mybir
    import concourse.tile as tile
    from concourse.bass_utils import run_bass_kernel_spmd

    _CACHE["mods"] = (bass, mybir, tile, run_bass_kernel_spmd)
    return _CACHE["mods"]


def _split_multiwaits(nc, mybir, max_waits=1):
    """The walrus build here rejects >1 sem-wait on some instructions (the
    Tile tail InstDrain, DMA_DIRECT2D). Hoist extra waits onto single-wait
    nops placed immediately before, on the same engine (same-engine program
    order preserves semantics)."""
    for fn in nc.m.functions:
        for bb in fn.blocks:
            insts = bb.instructions
            i = 0
            while i < len(insts):
                inst = insts[i]
                si = inst.sync_info
                if si is not None and si.on_wait and len(si.on_wait) > max_waits:
                    waits = list(si.on_wait)
                    keep = waits[-max_waits:]
                    for k, w in enumerate(waits[:-max_waits]):
                        nop = mybir.InstNoOp(
                            name=f"{inst.name}_hoistwait_{k}",
                            sync_info=mybir.SyncInfo(on_wait=[w], on_update=[]),
                            bass_nofuse=True,
                            engine=inst.engine,
                        )
                        insts.insert(i, nop)
                        i += 1
                    si.on_wait = keep
                i += 1


def _build_program():
    """Per-core inputs
      tissue (256, 256*64) bf16 channel-last (sample j, channel half h),
      cellrows (128,16,65,9) bf16 (plane-col slice for avg cols [8c,8c+8)),
      w6t (128,36,128) bf16 (full), bias (128,1) f32 (full),
      roff (1,3) i32 = [row0, col0, 512*h]
    output: out (256, 256*64) bf16 = tissue with avg added in the ROI."""
    if "nc" in _CACHE:
        return _CACHE["nc"]
    bass, mybir, tile, _ = _get_modules()
    f32, bf16, i32 = mybir.dt.float32, mybir.dt.bfloat16, mybir.dt.int32

    nc = bass.Bass("TRN2", target_bir_lowering=False, debug=False,
                   num_devices=NCORES)
    tissue_d = nc.dram_tensor("tissue", (H, W * CH), bf16, kind="ExternalInput").ap()
    cellrows_d = nc.dram_tensor("cellrows", (C, PHASES, PRR, KPR), bf16,
                                kind="ExternalInput").ap()
    w6t_d = nc.dram_tensor("w6t", (C, 36, C), bf16, kind="ExternalInput").ap()
    bias_d = nc.dram_tensor("bias", (C, 1), f32, kind="ExternalInput").ap()
    roff_d = nc.dram_tensor("roff", (1, 3), i32, kind="ExternalInput").ap()
    out_d = nc.dram_tensor("out", (H, W * CH), bf16, kind="ExternalOutput").ap()

    # taps grouped by polyphase plane group so matmuls chase the tile DMAs
    tap_order = []       # (tap_idx, plane, row_shift, col_shift)
    for pp in range(4):
        for qq in range(4):
            for p in range(pp, 6, 4):
                for q in range(qq, 6, 4):
                    tap_order.append((p * 6 + q, pp * 4 + qq, p // 4, q // 4))
    assert len(tap_order) == 36

    with tile.TileContext(nc) as tc:
        with (
            tc.tile_pool(name="const", bufs=1) as constp,
            tc.tile_pool(name="cellp", bufs=1) as cellp,
            tc.tile_pool(name="roip", bufs=1) as roip,
            tc.tile_pool(name="copyp", bufs=3) as copyp,
            tc.tile_pool(name="avgp", bufs=1) as avgp,
            tc.tile_pool(name="dram", bufs=1, space="DRAM") as dramp,
            tc.tile_pool(name="psum", bufs=1, space="PSUM") as psump,
        ):
            # --- tiny constants on gpsimd; weights first on scalar ---
            roff_sb = constp.tile([1, 3], i32)
            nc.gpsimd.dma_start(roff_sb[:], roff_d[:])
            bias_sb = constp.tile([C, 1], f32)
            nc.gpsimd.dma_start(bias_sb[:], bias_d[:])
            w_sb = constp.tile([C, 36 * C], bf16)
            nc.scalar.dma_start(w_sb[:], w6t_d[:])

            # --- cell planes: 4 plane-group tiles, 2 per HWDGE queue,
            # ahead of the bulk copy ---
            cr_ts = []
            cd4 = cellrows_d.rearrange("c (g p) r w -> g c (p r w)", g=4)
            for g in range(4):
                crt = cellp.tile([C, (PHASES // 4) * PRR * KPR], bf16,
                                 name=f"cr{g}")
                (nc.scalar if g < 2 else nc.sync).dma_start(crt[:], cd4[g])
                cr_ts.append(crt.rearrange("c (ph r w) -> c ph r w",
                                           r=PRR, w=KPR))
            zero_sb = constp.tile([C, 512], f32)
            nc.gpsimd.memset(zero_sb[:], 0.0)

            dyn_engines = (mybir.EngineType.SP, mybir.EngineType.Activation,
                           mybir.EngineType.Pool)
            r_v = nc.values_load(roff_sb[0:1, 0:1], engines=dyn_engines,
                                 min_val=0, max_val=H - ROI,
                                 skip_runtime_bounds_check=True)
            c_v = nc.values_load(roff_sb[0:1, 1:2], engines=dyn_engines,
                                 min_val=0, max_val=W - ROI,
                                 skip_runtime_bounds_check=True)
            h_v = nc.values_load(roff_sb[0:1, 2:3], engines=dyn_engines,
                                 min_val=0, max_val=CH * BCOLS,
                                 skip_runtime_bounds_check=True)

            tissue3 = tissue_d.rearrange("h (w c) -> h w c", c=CH)
            out3 = out_d.rearrange("h (w c) -> h w c", c=CH)

            # ROI source pixels, [128, 2048] layout (partition = r + 64*b,
            # b = column half): read early on scalar's ring
            roi_src = tissue3[bass.ds(r_v, ROI), bass.ds(c_v, ROI), :]
            roiT_sb = roip.tile([2 * ROI, (ROI // 2) * CH], bf16)
            for b in range(2):
                nc.scalar.dma_start(
                    roiT_sb[64 * b:64 * b + 64].rearrange(
                        "r (w c) -> r w c", c=CH),
                    roi_src[:, 32 * b:32 * b + 32, :],
                )

            # --- bulk copy tissue -> out through SBUF, bf16 ---
            CK = (W * CH) // 2
            for s in range(2):
                for k in range(2):
                    ct = copyp.tile([128, CK], bf16, tag="cp")
                    nc.scalar.dma_start(
                        ct[:], tissue_d[s * 128:(s + 1) * 128,
                                        k * CK:(k + 1) * CK])
                    nc.sync.dma_start(
                        out_d[s * 128:(s + 1) * 128, k * CK:(k + 1) * CK],
                        ct[:])

            # --- conv: full-channel avg cols [8c, 8c+8), 36 matmuls into
            # one PSUM bank, full 128-wide array ---
            ps = psump.tile([C, 64 * BCOLS], f32, name="bank0")
            for i, (t, ph, pb, qb) in enumerate(tap_order):
                nc.tensor.matmul(
                    ps[:],
                    w_sb[:, t * C:(t + 1) * C],
                    cr_ts[ph // 4][:, ph % 4, pb:pb + 64, qb:qb + BCOLS],
                    start=(i == 0),
                    stop=(i == 35),
                )

            # psum -> SBUF -> bf16; bias folded in (DVE)
            contrib_sb = avgp.tile([C, 64 * BCOLS], bf16)
            nc.vector.scalar_tensor_tensor(
                contrib_sb[:], ps[:], bias_sb[:], zero_sb[:],
                mybir.AluOpType.add, mybir.AluOpType.add,
            )

            # transpose [ch, (r64, c8)] -> contribT [r, (h2, c8, chh64)]
            # (bounce layout groups each channel half contiguously so the
            # dynamic-half readback is one 512-elem slice per block)
            contribT_sb = avgp.tile([ROI, 2 * BCOLS * CH], bf16)
            cv = contrib_sb.rearrange("p (r c) -> p c r", c=BCOLS)
            atv = contribT_sb.rearrange("p (h c q) -> p h c q", h=2, c=BCOLS)
            for bi in range(4):      # source ch 32-block
                for bj in range(2):  # source row 32-block
                    nc.vector.transpose(
                        atv[32 * bj:32 * bj + 32, bi // 2, :,
                            32 * (bi % 2):32 * (bi % 2) + 32],
                        cv[32 * bi:32 * bi + 32, :, 32 * bj:32 * bj + 32],
                    )
            bounce_in = dramp.tile([ROI, 2 * BCOLS * CH], bf16)
            nc.gpsimd.dma_start(bounce_in[:], contribT_sb[:])

            # --- AllGather the 8 column blocks across all cores (bf16) ---
            bounce_out = dramp.tile([8 * ROI, 2 * BCOLS * CH], bf16)
            nc.gpsimd.collective_compute(
                "AllGather",
                mybir.AluOpType.bypass,
                replica_groups=[[0, 1, 2, 3, 4, 5, 6, 7]],
                ins=[bounce_in[:].opt()],
                outs=[bounce_out[:].opt()],
            )
            # readback block c, this core's channel half (dynamic h_v):
            # -> avgT [128, 2048], partition = r + 64*(w>>5),
            #    free = (w&31)*64 + chh
            # one readback per HWDGE ring, split by partition half
            # (sync: blocks 0-3 -> partitions 0-63; scalar: 4-7 -> 64-127),
            # and the DVE add runs per half so each ROI write chases its own
            avgT_sb = avgp.tile([2 * ROI, (ROI // 2) * CH], bf16)
            bo_v = bounce_out.rearrange("(n p) f -> p n f", n=8)
            for half, eng in ((0, nc.sync), (1, nc.scalar)):
                eng.dma_start(
                    avgT_sb[64 * half:64 * half + 64].rearrange(
                        "p (k f) -> p k f", k=4),
                    bo_v[:, 4 * half:4 * half + 4, bass.ds(h_v, CH * BCOLS)])
            # single full-width add: DVE time scales with the free dim
            # (2048 cycles), so partition-split halves just serialize
            nc.vector.scalar_tensor_tensor(
                roiT_sb[:], avgT_sb[:], 0.0, roiT_sb[:],
                mybir.AluOpType.add, mybir.AluOpType.add,
            )

            # --- ROI overwrite: both writes on sync's ring, queued
            # after its bulk stores — the FIFO ring orders stores before
            # the overwrite without an all-engine barrier (and without
            # the ~3 us second-engine skew) ---
            roi_dst = out3[bass.ds(r_v, ROI), bass.ds(c_v, ROI), :]
            for b in range(2):
                nc.sync.dma_start(
                    roi_dst[:, 32 * b:32 * b + 32, :],
                    roiT_sb[64 * b:64 * b + 64].rearrange(
                        "r (w c) -> r w c", c=CH),
                )

    _split_multiwaits(nc, mybir)
    _CACHE["nc"] = nc
    return nc


def _prep_inputs(tissue_features, cell_features, loc, conv_w, conv_b):
    import ml_dtypes

    bf16 = ml_dtypes.bfloat16
    # fold AvgPool4x4 into the conv kernel: 6x6 taps
    w6 = np.zeros((C, C, 6, 6), np.float32)
    for dr in range(4):
        for dc in range(4):
            w6[:, :, dr:dr + 3, dc:dc + 3] += conv_w
    w6 *= 1.0 / 16.0

    # polyphase split of the zero-padded cell map:
    # plane (pp,qq)[y,x] = padded[4y+pp, 4x+qq], padded = 1px zero border
    padc = np.zeros((C, 4 * PRC, 4 * PRC), np.float32)
    padc[:, 1:1 + H, 1:1 + W] = cell_features[0]
    cell_poly = np.empty((C, PHASES, PRR, PRC), np.float32)
    for pp in range(4):
        for qq in range(4):
            cell_poly[:, pp * 4 + qq] = padc[:, pp:pp + 4 * PRR:4, qq::4]
    cell_poly = cell_poly.astype(bf16)

    # (C, C, 6, 6) -> (in ch, tap, out ch), full channel set
    w6t = np.ascontiguousarray(
        w6.transpose(1, 2, 3, 0).reshape(C, 36, C)).astype(bf16)
    bias = np.ascontiguousarray(conv_b.astype(np.float32)).reshape(C, 1)

    r0 = loc[:, 1].astype(np.int64) * W // 1024 - L   # H-dim start (from loc x)
    c0 = loc[:, 0].astype(np.int64) * W // 1024 - L   # W-dim start (from loc y)

    in_maps = []
    for c in range(NCORES):
        j, h = c % B, c // B
        # channel-last [H, W*CH] bf16 layout for tissue
        thwc = np.ascontiguousarray(
            tissue_features[j, CH * h:CH * (h + 1)].transpose(1, 2, 0)
        ).astype(bf16).reshape(H, W * CH)
        in_maps.append({
            "tissue": thwc,
            "cellrows": np.ascontiguousarray(
                cell_poly[:, :, :, BCOLS * c:BCOLS * c + KPR]),
            "w6t": w6t,
            "bias": bias,
            "roff": np.array([[r0[j], c0[j], CH * BCOLS * h]], np.int32),
        })
    return in_maps


def run_device(tissue_features, cell_features, loc, conv_w, conv_b, **spmd_kwargs):
    """Build+run the SPMD kernel; returns (final (4,128,256,256), raw results)."""
    *_, run_bass_kernel_spmd = _get_modules()
    nc = _build_program()
    in_maps = _prep_inputs(tissue_features, cell_features, loc, conv_w, conv_b)
    res = run_bass_kernel_spmd(nc, in_maps, list(range(NCORES)), **spmd_kwargs)
    final = np.empty((B, C, H, W), np.float32)
    for c in range(NCORES):
        j, h = c % B, c // B
        final[j, CH * h:CH * (h + 1)] = (
            res.results[c]["out"].astype(np.float32)
            .reshape(H, W, CH).transpose(2, 0, 1))
    return final, res


def kernel(tissue_features, cell_features, loc, conv_w, conv_b):
    final, _ = run_device(tissue_features, cell_features, loc, conv_w, conv_b)
    # reference stacks B copies of the fully-mutated tissue
    return np.broadcast_to(final[None], (B, B, C, H, W))
